# revision 22
# baseline (speedup 1.0000x reference)
"""Trainium2 Bass kernel for nn_Net_52218212384916.

Math identical to the previous revision (sample-point collapse of the two
conv_transpose stages; see git history / kernel_baseline.py).  This revision
restructures the device program for latency:

  - no mid-program DMAs: kernel-row reshapes ([1,225] -> [3,75]) are done on
    the PE via an ex-column transpose + 3 value matmuls at partition offsets
  - softmax uses the row form only, normalizes ex by 1/Z up front, so the
    downstream kernel rows need no deferred scaling
  - the 8 per-block key sigmoids are batched into one [128,24] PSUM tile and
    one tanh activation (sigmoid(x) = 0.5 tanh(x/2) + 0.5; tanh shares the
    ACT table with exp, so the only table switch is the final sigmoid, which
    lands in an idle ACT window)
  - stage F is 5 chunked fp32r matmuls (1 cycle/row at >=256 cols) into two
    stacked [96,512] PSUM banks + one small bank, 3 sigmoid passes, 2 output
    DMAs
"""
import numpy as np

H0 = 1024
S1 = (H0 - 5) // 2 + 1          # 510   conv1 output size
O1 = 2 * H0 - 1                 # 2047  out1 size
S2 = (O1 - 5) // 2 + 1          # 1022  conv2 output size
O2 = 2 * O1 - 1                 # 4093  out2 size
NCORES = 8
CHUNK = 512

_nc_cache = {}


# ---------------------------------------------------------------------------
# static structure (shapes only)
# ---------------------------------------------------------------------------

def _static():
    st = {}
    r1 = np.arange(32) * S1 // 32
    r2 = np.arange(32) * S2 // 32
    rf = np.arange(128) * O2 // 128
    a = -(-(rf - 2) // 2)            # first contributing out1 row
    gy = -(-(a - 2) // 2)            # first contributing x0 row
    e = a - 2 * gy                   # phase in {1,2}
    delta = (e == 2).astype(int)
    f = np.where(rf % 2 == 0, 2, 1)
    dim_type = np.empty(128, int)
    tmap = {(0, 2): 0, (1, 1): 1, (1, 2): 2, (0, 1): 3}
    for i in range(128):
        dim_type[i] = 4 if i == 0 else tmap[(delta[i], f[i])]
    st.update(r1=r1, r2=r2, rf=rf, a=a, gy=gy, dim_type=dim_type)
    st['dtype_delta'] = {0: 0, 1: 1, 2: 1, 3: 0, 4: 0}
    st['dtype_f'] = {0: 0, 1: 1, 2: 0, 3: 1, 4: 2}

    cls = dim_type[:, None] * 5 + dim_type[None, :]
    order = np.argsort(cls.ravel(), kind='stable')
    counts = np.bincount(cls.ravel(), minlength=25)
    Q = -(-counts // NCORES)
    offs = np.concatenate([[0], np.cumsum(Q)]).astype(int)
    NF = int(offs[-1])
    used = [k for k in range(25) if counts[k] > 0]
    pix_of_slot = -np.ones((NCORES, NF), np.int64)
    cstart = np.concatenate([[0], np.cumsum(counts)])
    for k in used:
        plist = order[cstart[k]:cstart[k + 1]]
        for c in range(NCORES):
            seg = plist[c * Q[k]:(c + 1) * Q[k]]
            pix_of_slot[c, offs[k]:offs[k] + len(seg)] = seg
    st.update(counts=counts, Q=Q, offs=offs, NF=NF, used=used,
              pix_of_slot=pix_of_slot)
    # slot -> used-class index (for output row addressing)
    kidx = np.zeros(NF, int)
    for ki, k in enumerate(used):
        kidx[offs[k]:offs[k] + Q[k]] = ki
    st['kidx'] = kidx
    return st


_ST = _static()
NF = _ST['NF']
NCH = -(-NF // CHUNK)            # 5 column chunks in stage F


# ---------------------------------------------------------------------------
# host-side gathers (raw values only; OOB -> 0.5)
# ---------------------------------------------------------------------------

def _gather_patches(img, row0s, col0s, n):
    C, H, W = img.shape
    R = row0s[:, None] + np.arange(n)[None, :]
    Cc = col0s[:, None] + np.arange(n)[None, :]
    vr, vc = (R >= 0) & (R < H), (Cc >= 0) & (Cc < W)
    Rc, Ccc = np.clip(R, 0, H - 1), np.clip(Cc, 0, W - 1)
    out = img[:, Rc[:, None, :, None], Ccc[None, :, None, :]]
    mask = vr[:, None, :, None] & vc[None, :, None, :]
    out = np.where(mask[None], out, np.float32(0.5))
    C_, NI, NJ, n_, _ = out.shape
    return np.ascontiguousarray(
        out.transpose(0, 3, 4, 1, 2).reshape(C_ * n_ * n_, NI * NJ), np.float32)


def _prep(ins, st):
    img = np.asarray(ins['input'], np.float32)[0]
    r1, r2, gy = st['r1'], st['r2'], st['gy']
    d = {}
    xp1 = _gather_patches(img, 2 * r1, 2 * r1, 5)              # [75,1024]
    d['xp1aug'] = np.concatenate(
        [xp1, np.full((1, 1024), 1.0, np.float32)], 0)         # [76,1024]
    x0p2 = _gather_patches(img, r2 - 1, r2 - 1, 5)             # [75,1024]
    d['x0p2'] = np.concatenate(
        [x0p2, np.full((1, 1024), 1.0, np.float32)], 0)        # [76,1024]
    w1 = np.asarray(ins['lk1_conv_w'], np.float32)             # [oc,ic,5,5]
    b1 = np.asarray(ins['lk1_conv_b'], np.float32)
    # K1 weights: rows (ic,ky,kx)+bias ; K2 weights: rows (ky,kx,c)+bias
    wa = w1.transpose(1, 2, 3, 0).reshape(75, 3)
    wb = w1.transpose(2, 3, 1, 0).reshape(75, 3)
    d['w1a'] = np.concatenate([wa, b1[None]], 0).astype(np.float32)
    d['w1b'] = np.concatenate([wb, b1[None]], 0).astype(np.float32)
    keys = np.asarray(ins['lk1_keys'], np.float32)             # [100,3072]
    keysR = np.ascontiguousarray(
        keys.T.reshape(24, 128, 100).transpose(1, 0, 2), np.float32
    ).reshape(128, 2400)
    d['keysA'] = np.ascontiguousarray(keysR[:, :1200])
    d['keysB'] = np.ascontiguousarray(keysR[:, 1200:])
    # values with columns permuted (in,out,ky,kx) -> (in,ky,kx,out)
    vals = np.asarray(ins['lk1_values'], np.float32)
    d['valsP'] = np.ascontiguousarray(
        vals.reshape(100, 3, 3, 5, 5).transpose(0, 1, 3, 4, 2)
    ).reshape(100, 225)

    # T'' selection [3, 25*75]: sall[ic, uv*75 + ic*25 + uv] = 1
    sall = np.zeros((3, 1875), np.float32)
    for ic in range(3):
        for uv in range(25):
            sall[ic, uv * 75 + ic * 25 + uv] = 1.0
    d['sall'] = sall
    # T' selection [3, 16*48]: s48[ic, uv*48 + uv*3 + ic] = 1  (rows (u,v,ic))
    s48 = np.zeros((3, 768), np.float32)
    for ic in range(3):
        for uv in range(16):
            s48[ic, uv * 48 + uv * 3 + ic] = 1.0
    d['s48'] = s48
    # F (s,t) selection [3, 9*27]: sst[c, st*27 + c*9 + st] = 1
    sst = np.zeros((3, 243), np.float32)
    for c in range(3):
        for stx in range(9):
            sst[c, stx * 27 + c * 9 + stx] = 1.0
    d['sst'] = sst
    d['ident'] = np.eye(100, dtype=np.float32)
    # aux row: [bias(3) | e75 one-hot(76)] for the wk2 bias outer-product
    aux = np.zeros((1, 79), np.float32)
    aux[0, 0:3] = b1
    aux[0, 3 + 75] = 1.0
    d['aux'] = aux

    # stage F windows, per core
    pix = st['pix_of_slot']
    uu = np.arange(4)
    x0w = []
    for c in range(NCORES):
        p = pix[c]
        ii, jj = p // 128, p % 128
        R = gy[np.clip(ii, 0, 127)][:, None] + uu[None, :]
        Cc = gy[np.clip(jj, 0, 127)][:, None] + uu[None, :]
        ok = (p >= 0)[:, None]
        vr = (R >= 0) & (R < H0) & ok
        vc = (Cc >= 0) & (Cc < H0) & ok
        Rc, Ccc = np.clip(R, 0, H0 - 1), np.clip(Cc, 0, H0 - 1)
        g = img[:, Rc[:, :, None], Ccc[:, None, :]]            # [3,NF,4,4]
        m = vr[:, :, None] & vc[:, None, :]
        g = np.where(m[None], g, np.float32(0.5))
        # row order (u, v, ic) to match M4T/W layout
        x0w.append(np.ascontiguousarray(
            g.transpose(2, 3, 0, 1).reshape(48, NF), np.float32))
    return d, x0w


# ---------------------------------------------------------------------------
# device program
# ---------------------------------------------------------------------------

def _build_nc():
    import concourse.bacc as bacc
    import concourse.tile as tile
    from concourse import mybir

    F32 = mybir.dt.float32
    F32R = mybir.dt.float32r
    BF16 = mybir.dt.bfloat16
    AF = mybir.ActivationFunctionType
    ALU = mybir.AluOpType
    AX = mybir.AxisListType
    st = _ST

    nc = bacc.Bacc("TRN2", target_bir_lowering=False, debug=False)
    t_xp1 = nc.dram_tensor("xp1aug", [76, 1024], F32, kind="ExternalInput")
    t_xp2 = nc.dram_tensor("x0p2", [76, 1024], F32, kind="ExternalInput")
    t_w1a = nc.dram_tensor("w1a", [76, 3], F32, kind="ExternalInput")
    t_w1b = nc.dram_tensor("w1b", [76, 3], F32, kind="ExternalInput")
    t_keysA = nc.dram_tensor("keysA", [128, 1200], F32, kind="ExternalInput")
    t_keysB = nc.dram_tensor("keysB", [128, 1200], F32, kind="ExternalInput")
    t_vals = nc.dram_tensor("valsP", [100, 225], F32, kind="ExternalInput")
    t_sall = nc.dram_tensor("sall", [3, 1875], F32, kind="ExternalInput")
    t_sst = nc.dram_tensor("sst", [3, 243], F32, kind="ExternalInput")
    t_s48 = nc.dram_tensor("s48", [3, 768], F32, kind="ExternalInput")
    t_ident = nc.dram_tensor("ident", [100, 100], F32, kind="ExternalInput")
    t_aux = nc.dram_tensor("aux", [1, 79], F32, kind="ExternalInput")
    t_x0w = nc.dram_tensor("x0w", [48, NF], F32, kind="ExternalInput")
    t_out96a = nc.dram_tensor("out96a", [112, CHUNK], BF16, kind="ExternalOutput")
    t_out96b = nc.dram_tensor("out96b", [112, CHUNK], BF16, kind="ExternalOutput")
    t_out2 = nc.dram_tensor("out2", [48, 16], BF16, kind="ExternalOutput")

    with tile.TileContext(nc) as tc:
        with tc.tile_pool(name="sb", bufs=1) as sb, \
             tc.tile_pool(name="sbc", bufs=4) as sbc, \
             tc.tile_pool(name="psA", bufs=1, space="PSUM") as psA, \
             tc.tile_pool(name="psB", bufs=1, space="PSUM") as psB, \
             tc.tile_pool(name="psF", bufs=1, space="PSUM") as psF:

            # ---- loads (all issued up front; none mid-chain)
            xp1_sb = sb.tile([76, 1024], F32)
            xp2_sb = sb.tile([76, 1024], F32)
            w1a_sb = sb.tile([76, 3], F32)
            w1b_sb = sb.tile([76, 3], F32)
            keysA_sb = sb.tile([128, 1200], F32)
            keysB_sb = sb.tile([128, 1200], F32)
            vals_sb = sb.tile([100, 225], F32)
            sall_sb = sb.tile([3, 1875], F32)
            sst_sb = sb.tile([3, 243], F32)
            s48_sb = sb.tile([3, 768], F32)
            ident_sb = sb.tile([100, 100], F32)
            aux_sb = sb.tile([1, 79], F32)
            x0w_sb = sb.tile([48, NF], F32)
            for eng, tdst, tsrc in [
                    (nc.sync, xp1_sb, t_xp1),
                    (nc.gpsimd, w1a_sb, t_w1a),
                    (nc.scalar, keysA_sb, t_keysA),
                    (nc.gpsimd, ident_sb, t_ident),
                    (nc.scalar, keysB_sb, t_keysB),
                    (nc.sync, xp2_sb, t_xp2),
                    (nc.gpsimd, vals_sb, t_vals),
                    (nc.scalar, sall_sb, t_sall),
                    (nc.gpsimd, w1b_sb, t_w1b),
                    (nc.scalar, s48_sb, t_s48),
                    (nc.gpsimd, sst_sb, t_sst),
                    (nc.scalar, aux_sb, t_aux),
                    (nc.sync, x0w_sb, t_x0w)]:
                eng.dma_start(tdst[:], tsrc[:])

            ones100 = sb.tile([1, 100], F32)
            nc.gpsimd.memset(ones100[:], 1.0)
            out2_sb = sb.tile([48, 16], BF16)
            nc.gpsimd.memset(out2_sb[:], 0.0)

            # ---- elementwise input prep (2x-1), chunked for pipelining
            xa = sb.tile([76, 1024], F32)
            for xh in range(4):
                nc.vector.tensor_scalar(xa[:, xh * 256:(xh + 1) * 256],
                                        xp1_sb[:, xh * 256:(xh + 1) * 256],
                                        2.0, -1.0,
                                        op0=ALU.mult, op1=ALU.add)
            kvA = keysA_sb.rearrange("p (cc k) -> p cc k", k=100)
            kvB = keysB_sb.rearrange("p (cc k) -> p cc k", k=100)

            # ---------------- key/attention stage (shared emitter)
            # returns normalized kernel row as [3, 75] (rows ic / c,
            # cols (k_t, k_tx, other-channel))
            def key_stage(xaug_sb, w_sb, tag):
                # conv keys: one batched [128, 24] PSUM tile
                pk = psA.tile([128, 24], F32, tag="pk")
                for m in range(8):
                    nc.tensor.matmul(pk[:, m * 3:m * 3 + 3],
                                     xaug_sb[:, m * 128:(m + 1) * 128],
                                     w_sb[:], start=True, stop=True,
                                     skip_group_check=True)
                # sigmoid(x) = 0.5*tanh(x/2) + 0.5  (tanh shares exp's table)
                th = sbc.tile([128, 24], F32, tag="th")
                nc.scalar.activation(th[:], pk[:], AF.Tanh, scale=0.5)
                keyT = sb.tile([128, 24], F32, tag=f"keyT{tag}")
                nc.vector.tensor_scalar(keyT[:], th[:], 0.5, 0.5,
                                        op0=ALU.mult, op1=ALU.add)
                # logits column via one accumulated contraction pass
                lc0 = psB.tile([100, 1], F32, tag="acc")
                for oc in range(3):
                    for m in range(8):
                        cc = oc * 8 + m
                        kvh = kvA[:, cc, :] if cc < 12 else kvB[:, cc - 12, :]
                        nc.tensor.matmul(
                            lc0[:], kvh,
                            keyT[:, m * 3 + oc:m * 3 + oc + 1],
                            start=(cc == 0), stop=(cc == 23))
                lc0_sb = sb.tile([100, 1], F32, tag=f"lc0{tag}")
                nc.vector.tensor_copy(lc0_sb[:], lc0[:])
                # row view via PE transpose (matmul against identity)
                lrT = psA.tile([1, 100], F32, tag="pk")
                nc.tensor.matmul(lrT[:], lc0_sb[:], ident_sb[:],
                                 start=True, stop=True)
                mx = sb.tile([1, 1], F32, tag=f"mx{tag}")
                nc.vector.reduce_max(mx[:], lrT[:], axis=AX.X)
                negm = sb.tile([1, 1], F32, tag=f"negm{tag}")
                nc.vector.tensor_scalar_mul(negm[:], mx[:], -1.0)
                ex = sb.tile([1, 100], F32, tag=f"ex{tag}")
                Z = sb.tile([1, 1], F32, tag=f"Z{tag}")
                nc.scalar.activation(ex[:], lrT[:], AF.Exp, bias=negm[:],
                                     accum_out=Z[:])
                rz = sb.tile([1, 1], F32, tag=f"rz{tag}")
                nc.vector.reciprocal(rz[:], Z[:])
                # attention column [100,1] via 1-partition PE transpose;
                # rhs = 1/Z folds the softmax normalization into the same op
                exc = psA.tile([100, 1], F32, tag="pk")
                nc.tensor.matmul(exc[:], ex[:], rz[:],
                                 start=True, stop=True)
                exc_sb = sb.tile([100, 1], F32, tag=f"exc{tag}")
                nc.vector.tensor_copy(exc_sb[:], exc[:])
                # kernel row as [75, 3] (column writes are offset-free),
                # then PE-transpose to the consumer layout [3, 75]
                krT = psB.tile([75, 3], F32, tag="acc")
                for ic in range(3):
                    nc.tensor.matmul(krT[:, ic:ic + 1],
                                     vals_sb[:, ic * 75:(ic + 1) * 75],
                                     exc_sb[:], start=True, stop=True,
                                     skip_group_check=True)
                krT_sb = sb.tile([75, 3], F32, tag=f"krT{tag}")
                nc.vector.tensor_copy(krT_sb[:], krT[:])
                krp = psA.tile([3, 75], F32, tag="pk")
                nc.tensor.matmul(krp[:], krT_sb[:], ident_sb[0:75, 0:75],
                                 start=True, stop=True)
                kresh = sb.tile([3, 75], F32, tag=f"kresh{tag}")
                nc.vector.tensor_copy(kresh[:], krp[:])
                return kresh

            # ---------------- stage K1
            k1resh = key_stage(xa, w1a_sb, "1")

            xm2 = sb.tile([76, 1024], F32)
            for xh in range(2):
                nc.vector.tensor_scalar(xm2[:, xh * 512:(xh + 1) * 512],
                                        xp2_sb[:, xh * 512:(xh + 1) * 512],
                                        2.0, -1.0,
                                        op0=ALU.mult, op1=ALU.add)

            # ---------------- T'' via 25 accumulated selection matmuls
            # T''[(ic,u,v),(ky,kx,c)] = k1[ic,c,ky+4-2u,kx+4-2v]
            k1rv = k1resh.rearrange("ic (kt ktx c) -> ic kt ktx c",
                                    kt=5, ktx=5)
            tpps = psB.tile([75, 75], F32, tag="bld")
            tppsv = tpps.rearrange("p (ky kx c) -> p ky kx c", ky=5, kx=5)
            uvs = [(2, 2)] + [(u, v) for u in range(5) for v in range(5)
                              if (u, v) != (2, 2)]
            for i, (u, v) in enumerate(uvs):
                klo, khi = max(0, 2 * u - 4), min(4, 2 * u)
                xlo, xhi = max(0, 2 * v - 4), min(4, 2 * v)
                nc.tensor.matmul(
                    tppsv[:, klo:khi + 1, xlo:xhi + 1, :],
                    sall_sb[:, (u * 5 + v) * 75:(u * 5 + v + 1) * 75],
                    k1rv[:, klo + 4 - 2 * u:khi + 5 - 2 * u,
                         xlo + 4 - 2 * v:xhi + 5 - 2 * v, :],
                    start=(i == 0), stop=(i == len(uvs) - 1),
                    skip_group_check=True)
            tpp_sb = sb.tile([75, 75], F32)
            nc.vector.tensor_copy(tpp_sb[:], tpps[:])

            # ---------------- compose K2 weights: WK2 = [T'' @ w1b75 ; b]
            ptp = psB.tile([75, 75], F32, tag="bld")
            nc.tensor.matmul(ptp[:], tpp_sb[:], ident_sb[0:75, 0:75],
                             start=True, stop=True)
            tppT_sb = sb.tile([75, 75], F32)
            nc.vector.tensor_copy(tppT_sb[:], ptp[:])
            pwk = psB.tile([76, 3], F32, tag="bld")
            # rank-1 bias row first (start zeroes all 76 rows), then the
            # weight part accumulates rows 0..74
            nc.tensor.matmul(pwk[:], aux_sb[:, 3:79], aux_sb[:, 0:3],
                             start=True, stop=False, skip_group_check=True)
            nc.tensor.matmul(pwk[0:75, :], tppT_sb[:], w1b_sb[0:75, :],
                             start=False, stop=True, skip_group_check=True)
            wk2_sb = sb.tile([76, 3], F32)
            nc.vector.tensor_copy(wk2_sb[:], pwk[:])

            # xwm prep (needed only by stage F; DVE is free here)
            xwm = sb.tile([48, NF], BF16)
            for xc in range(4):
                lo = xc * ((NF + 3) // 4)
                hi = min(NF, (xc + 1) * ((NF + 3) // 4))
                nc.vector.tensor_scalar(xwm[:, lo:hi], x0w_sb[:, lo:hi],
                                        2.0, -1.0,
                                        op0=ALU.mult, op1=ALU.add)

            # ---------------- T' variants [48, 27] via selection matmuls
            # (emitted before K2 so the PE fills K2's latency gaps)
            tpv_sb = sb.tile([48, 108], F32)
            for dvi, (er_, ec_) in enumerate([(1, 1), (1, 2), (2, 1),
                                              (2, 2)]):
                def blocks(e):
                    bl = []
                    for u in range(4):
                        lo, hi = max(0, 2 * u - 2 - e), min(2, 2 * u + 2 - e)
                        if lo <= hi:
                            bl.append((u, lo, hi))
                    return bl
                ub, vb = blocks(er_), blocks(ec_)
                ub.sort(key=lambda b: -(b[2] - b[1]))
                vb.sort(key=lambda b: -(b[2] - b[1]))
                tps = psB.tile([48, 27], F32, tag="bld")
                tpsv = tps.rearrange("p (c s t) -> p s t c", s=3, t=3)
                nbl = len(ub) * len(vb)
                j = 0
                for u, slo, shi in ub:
                    ktlo = er_ + 2 + slo - 2 * u
                    for v, tlo, thi in vb:
                        ktxlo = ec_ + 2 + tlo - 2 * v
                        j += 1
                        nc.tensor.matmul(
                            tpsv[:, slo:shi + 1, tlo:thi + 1, :],
                            s48_sb[:, (u * 4 + v) * 48:(u * 4 + v + 1) * 48],
                            k1rv[:, ktlo:ktlo + shi - slo + 1,
                                 ktxlo:ktxlo + thi - tlo + 1, :],
                            start=(j == 1), stop=(j == nbl),
                            skip_group_check=True)
                nc.vector.tensor_copy(
                    tpv_sb[:, dvi * 27:(dvi + 1) * 27], tps[:])
            # transposed variants [27, 48] for the W composition
            tpvT_sb = sb.tile([27, 192], F32)
            for dvi in range(4):
                ptt = psA.tile([27, 48], F32, tag="pwf")
                nc.tensor.matmul(ptt[:],
                                 tpv_sb[:, dvi * 27:(dvi + 1) * 27],
                                 ident_sb[0:48, 0:48], start=True, stop=True)
                nc.vector.tensor_copy(
                    tpvT_sb[:, dvi * 48:(dvi + 1) * 48], ptt[:])

            # ---------------- stage K2
            k2resh = key_stage(xm2, wk2_sb, "2")

            # ---------------- F variants via selection matmuls
            srange = {0: (0, 3, 2), 1: (0, 2, 1), 2: (1, 3, 2)}
            k2rv = k2resh.rearrange("c (ky kx o) -> c ky kx o", ky=5, kx=5)
            fps = psB.tile([27, 27], F32, tag="bld")
            for vr_ in range(3):
                slo, shi, fy = srange[vr_]
                for vc_ in range(3):
                    tlo, thi, fx = srange[vc_]
                    vi = vr_ * 3 + vc_
                    sts = [(s, t) for s in range(slo, shi)
                           for t in range(tlo, thi)]
                    for j, (s, t) in enumerate(sts):
                        nc.tensor.matmul(
                            fps[:, vi * 3:vi * 3 + 3],
                            sst_sb[:, (s * 3 + t) * 27:(s * 3 + t + 1) * 27],
                            k2rv[:, fy + 2 - 2 * s, fx + 2 - 2 * t, :],
                            start=(j == 0), stop=(j == len(sts) - 1),
                            skip_group_check=True)
            f_sb = sb.tile([27, 27], F32)
            nc.vector.tensor_copy(f_sb[:], fps[:])

            # ---------------- W_k = T'varT.T @ F_var  -> [48, 48]
            dd, df = st['dtype_delta'], st['dtype_f']
            used = st['used']
            w_sb = sb.tile([48, 48], BF16)
            pwall = psA.tile([48, 48], F32, tag="pwf")
            for ki, k in enumerate(used):
                ta, tb = k // 5, k % 5
                dvi = dd[ta] * 2 + dd[tb]
                fvi = df[ta] * 3 + df[tb]
                nc.tensor.matmul(pwall[:, ki * 3:ki * 3 + 3],
                                 tpvT_sb[:, dvi * 48:(dvi + 1) * 48],
                                 f_sb[:, fvi * 3:fvi * 3 + 3],
                                 start=True, stop=True, skip_group_check=True)
            nc.vector.tensor_copy(w_sb[:], pwall[:])

            # ---------------- stage F: tmp = W^T @ xwm, chunked fp32r
            # chunk parity stacks at partition offset 64 (PE tile-position
            # rule); rows 48..63 are memset so the full-tile sigmoid stays
            # finite
            bankA = psF.tile([112, CHUNK], F32, tag="fa")
            bankB = psF.tile([112, CHUNK], F32, tag="fb")
            bankC = psF.tile([48, 16], F32, tag="fc")
            nc.vector.memset(bankA[32:64, :], 0.0)
            nc.vector.memset(bankB[32:64, :], 0.0)
            out96 = sb.tile([112, 1024], BF16)
            for ch in range(4):
                bank = bankA if ch < 2 else bankB
                po = 64 * (ch % 2)
                nc.tensor.matmul(bank[po:po + 48, :], w_sb[:],
                                 xwm[:, ch * CHUNK:(ch + 1) * CHUNK],
                                 start=True, stop=True,
                                 skip_group_check=True)
            nrem = NF - 4 * CHUNK
            nc.tensor.matmul(bankC[:, 0:nrem], w_sb[:],
                             xwm[:, 4 * CHUNK:NF],
                             start=True, stop=True, skip_group_check=True)
            nc.scalar.activation(out96[:, 0:CHUNK], bankA[:], AF.Sigmoid)
            nc.sync.dma_start(t_out96a[:], out96[:, 0:CHUNK])
            nc.scalar.activation(out96[:, CHUNK:1024], bankB[:], AF.Sigmoid)
            nc.scalar.activation(out2_sb[:, 0:nrem], bankC[:, 0:nrem],
                                 AF.Sigmoid)
            nc.gpsimd.dma_start(t_out2[:], out2_sb[:])
            nc.scalar.dma_start(t_out96b[:], out96[:, CHUNK:1024])
    nc.compile()
    return nc


# ---------------------------------------------------------------------------
# entry point
# ---------------------------------------------------------------------------

def _run(ins, trace=False):
    from concourse.bass_utils import run_bass_kernel_spmd
    if 'nc' not in _nc_cache:
        _nc_cache['nc'] = _build_nc()
    nc = _nc_cache['nc']
    d, x0w = _prep(ins, _ST)
    in_maps = [{**d, "x0w": x0w[c]} for c in range(NCORES)]
    res = run_bass_kernel_spmd(nc, in_maps, core_ids=list(range(NCORES)),
                               trace=trace)
    return res


def _assemble(results):
    st = _ST
    kidx = st['kidx']
    s = np.arange(NF)
    chunk = s // CHUNK
    final = np.zeros((3, 128, 128), np.float32)
    for c in range(NCORES):
        r = results[c]
        o96 = np.concatenate([np.asarray(r["out96a"], np.float32),
                              np.asarray(r["out96b"], np.float32)], axis=1)
        o2 = np.asarray(r["out2"], np.float32)
        vals = np.empty((3, NF), np.float32)
        for ch_ in range(3):
            row = 3 * kidx + ch_
            in96 = chunk < 4
            vals[ch_, in96] = o96[64 * (chunk[in96] % 2) + row[in96],
                                  CHUNK * (chunk[in96] // 2) + s[in96] % CHUNK]
            vals[ch_, ~in96] = o2[row[~in96], s[~in96] - 4 * CHUNK]
        pix = st['pix_of_slot'][c]
        valid = pix >= 0
        final[:, pix[valid] // 128, pix[valid] % 128] = vals[:, valid]
    return final[None]


def kernel(**inputs) -> np.ndarray:
    res = _run(inputs)
    return _assemble(res.results)


# revision 23
# speedup vs baseline: 1.0954x; 1.0954x over previous
"""Trainium2 Bass kernel for nn_Net_52218212384916.

Math identical to the previous revision (sample-point collapse of the two
conv_transpose stages; see git history / kernel_baseline.py).  This revision
restructures the device program for latency:

  - no mid-program DMAs: kernel-row reshapes ([1,225] -> [3,75]) are done on
    the PE via an ex-column transpose + 3 value matmuls at partition offsets
  - softmax uses the row form only, normalizes ex by 1/Z up front, so the
    downstream kernel rows need no deferred scaling
  - the 8 per-block key sigmoids are batched into one [128,24] PSUM tile and
    one tanh activation (sigmoid(x) = 0.5 tanh(x/2) + 0.5; tanh shares the
    ACT table with exp, so the only table switch is the final sigmoid, which
    lands in an idle ACT window)
  - stage F is 5 chunked fp32r matmuls (1 cycle/row at >=256 cols) into two
    stacked [96,512] PSUM banks + one small bank, 3 sigmoid passes, 2 output
    DMAs
"""
import numpy as np

H0 = 1024
S1 = (H0 - 5) // 2 + 1          # 510   conv1 output size
O1 = 2 * H0 - 1                 # 2047  out1 size
S2 = (O1 - 5) // 2 + 1          # 1022  conv2 output size
O2 = 2 * O1 - 1                 # 4093  out2 size
NCORES = 8
CHUNK = 512

_nc_cache = {}


# ---------------------------------------------------------------------------
# static structure (shapes only)
# ---------------------------------------------------------------------------

def _static():
    st = {}
    r1 = np.arange(32) * S1 // 32
    r2 = np.arange(32) * S2 // 32
    rf = np.arange(128) * O2 // 128
    a = -(-(rf - 2) // 2)            # first contributing out1 row
    gy = -(-(a - 2) // 2)            # first contributing x0 row
    e = a - 2 * gy                   # phase in {1,2}
    delta = (e == 2).astype(int)
    f = np.where(rf % 2 == 0, 2, 1)
    dim_type = np.empty(128, int)
    tmap = {(0, 2): 0, (1, 1): 1, (1, 2): 2, (0, 1): 3}
    for i in range(128):
        dim_type[i] = 4 if i == 0 else tmap[(delta[i], f[i])]
    st.update(r1=r1, r2=r2, rf=rf, a=a, gy=gy, dim_type=dim_type)
    st['dtype_delta'] = {0: 0, 1: 1, 2: 1, 3: 0, 4: 0}
    st['dtype_f'] = {0: 0, 1: 1, 2: 0, 3: 1, 4: 2}

    cls = dim_type[:, None] * 5 + dim_type[None, :]
    order = np.argsort(cls.ravel(), kind='stable')
    counts = np.bincount(cls.ravel(), minlength=25)
    Q = -(-counts // NCORES)
    offs = np.concatenate([[0], np.cumsum(Q)]).astype(int)
    NF = int(offs[-1])
    used = [k for k in range(25) if counts[k] > 0]
    pix_of_slot = -np.ones((NCORES, NF), np.int64)
    cstart = np.concatenate([[0], np.cumsum(counts)])
    for k in used:
        plist = order[cstart[k]:cstart[k + 1]]
        for c in range(NCORES):
            seg = plist[c * Q[k]:(c + 1) * Q[k]]
            pix_of_slot[c, offs[k]:offs[k] + len(seg)] = seg
    st.update(counts=counts, Q=Q, offs=offs, NF=NF, used=used,
              pix_of_slot=pix_of_slot)
    # slot -> used-class index (for output row addressing)
    kidx = np.zeros(NF, int)
    for ki, k in enumerate(used):
        kidx[offs[k]:offs[k] + Q[k]] = ki
    st['kidx'] = kidx
    return st


_ST = _static()
NF = _ST['NF']
NCH = -(-NF // CHUNK)            # 5 column chunks in stage F


# ---------------------------------------------------------------------------
# host-side gathers (raw values only; OOB -> 0.5)
# ---------------------------------------------------------------------------

def _gather_patches(img, row0s, col0s, n):
    C, H, W = img.shape
    R = row0s[:, None] + np.arange(n)[None, :]
    Cc = col0s[:, None] + np.arange(n)[None, :]
    vr, vc = (R >= 0) & (R < H), (Cc >= 0) & (Cc < W)
    Rc, Ccc = np.clip(R, 0, H - 1), np.clip(Cc, 0, W - 1)
    out = img[:, Rc[:, None, :, None], Ccc[None, :, None, :]]
    mask = vr[:, None, :, None] & vc[None, :, None, :]
    out = np.where(mask[None], out, np.float32(0.5))
    C_, NI, NJ, n_, _ = out.shape
    return np.ascontiguousarray(
        out.transpose(0, 3, 4, 1, 2).reshape(C_ * n_ * n_, NI * NJ), np.float32)


def _prep(ins, st):
    img = np.asarray(ins['input'], np.float32)[0]
    r1, r2, gy = st['r1'], st['r2'], st['gy']
    d = {}
    xp1 = _gather_patches(img, 2 * r1, 2 * r1, 5)              # [75,1024]
    d['xp1aug'] = np.concatenate(
        [xp1, np.full((1, 1024), 1.0, np.float32)], 0)         # [76,1024]
    x0p2 = _gather_patches(img, r2 - 1, r2 - 1, 5)             # [75,1024]
    d['x0p2'] = np.concatenate(
        [x0p2, np.full((1, 1024), 1.0, np.float32)], 0)        # [76,1024]
    w1 = np.asarray(ins['lk1_conv_w'], np.float32)             # [oc,ic,5,5]
    b1 = np.asarray(ins['lk1_conv_b'], np.float32)
    # K1 weights: rows (ic,ky,kx)+bias ; K2 weights: rows (ky,kx,c)+bias
    wa = w1.transpose(1, 2, 3, 0).reshape(75, 3)
    wb = w1.transpose(2, 3, 1, 0).reshape(75, 3)
    d['w1a'] = np.concatenate([wa, b1[None]], 0).astype(np.float32)
    d['w1b'] = np.concatenate([wb, b1[None]], 0).astype(np.float32)
    keys = np.asarray(ins['lk1_keys'], np.float32)             # [100,3072]
    keysR = np.ascontiguousarray(
        keys.T.reshape(24, 128, 100).transpose(1, 0, 2), np.float32
    ).reshape(128, 2400)
    d['keysA'] = np.ascontiguousarray(keysR[:, :1200]).astype(np.float16)
    d['keysB'] = np.ascontiguousarray(keysR[:, 1200:]).astype(np.float16)
    # values with columns permuted (in,out,ky,kx) -> (in,ky,kx,out)
    vals = np.asarray(ins['lk1_values'], np.float32)
    d['valsP'] = np.ascontiguousarray(
        vals.reshape(100, 3, 3, 5, 5).transpose(0, 1, 3, 4, 2)
    ).reshape(100, 225)

    # T'' selection [3, 25*75]: sall[ic, uv*75 + ic*25 + uv] = 1
    sall = np.zeros((3, 1875), np.float32)
    for ic in range(3):
        for uv in range(25):
            sall[ic, uv * 75 + ic * 25 + uv] = 1.0
    d['sall'] = sall
    # T' selection [3, 16*48]: s48[ic, uv*48 + uv*3 + ic] = 1  (rows (u,v,ic))
    s48 = np.zeros((3, 768), np.float32)
    for ic in range(3):
        for uv in range(16):
            s48[ic, uv * 48 + uv * 3 + ic] = 1.0
    d['s48'] = s48
    # F (s,t) selection [3, 9*27]: sst[c, st*27 + c*9 + st] = 1
    sst = np.zeros((3, 243), np.float32)
    for c in range(3):
        for stx in range(9):
            sst[c, stx * 27 + c * 9 + stx] = 1.0
    d['sst'] = sst
    d['ident'] = np.eye(100, dtype=np.float32)
    # aux row: [bias(3) | e75 one-hot(76)] for the wk2 bias outer-product
    aux = np.zeros((1, 79), np.float32)
    aux[0, 0:3] = b1
    aux[0, 3 + 75] = 1.0
    d['aux'] = aux

    # stage F windows, per core
    pix = st['pix_of_slot']
    uu = np.arange(4)
    x0w = []
    for c in range(NCORES):
        p = pix[c]
        ii, jj = p // 128, p % 128
        R = gy[np.clip(ii, 0, 127)][:, None] + uu[None, :]
        Cc = gy[np.clip(jj, 0, 127)][:, None] + uu[None, :]
        ok = (p >= 0)[:, None]
        vr = (R >= 0) & (R < H0) & ok
        vc = (Cc >= 0) & (Cc < H0) & ok
        Rc, Ccc = np.clip(R, 0, H0 - 1), np.clip(Cc, 0, H0 - 1)
        g = img[:, Rc[:, :, None], Ccc[:, None, :]]            # [3,NF,4,4]
        m = vr[:, :, None] & vc[:, None, :]
        g = np.where(m[None], g, np.float32(0.5))
        # row order (u, v, ic) to match M4T/W layout
        x0w.append(np.ascontiguousarray(
            g.transpose(2, 3, 0, 1).reshape(48, NF), np.float32))
    return d, x0w


# ---------------------------------------------------------------------------
# device program
# ---------------------------------------------------------------------------

def _build_nc():
    import concourse.bacc as bacc
    import concourse.tile as tile
    from concourse import mybir

    F32 = mybir.dt.float32
    F32R = mybir.dt.float32r
    BF16 = mybir.dt.bfloat16
    FP16 = mybir.dt.float16
    AF = mybir.ActivationFunctionType
    ALU = mybir.AluOpType
    AX = mybir.AxisListType
    st = _ST

    nc = bacc.Bacc("TRN2", target_bir_lowering=False, debug=False)
    t_xp1 = nc.dram_tensor("xp1aug", [76, 1024], F32, kind="ExternalInput")
    t_xp2 = nc.dram_tensor("x0p2", [76, 1024], F32, kind="ExternalInput")
    t_w1a = nc.dram_tensor("w1a", [76, 3], F32, kind="ExternalInput")
    t_w1b = nc.dram_tensor("w1b", [76, 3], F32, kind="ExternalInput")
    t_keysA = nc.dram_tensor("keysA", [128, 1200], FP16, kind="ExternalInput")
    t_keysB = nc.dram_tensor("keysB", [128, 1200], FP16, kind="ExternalInput")
    t_vals = nc.dram_tensor("valsP", [100, 225], F32, kind="ExternalInput")
    t_sall = nc.dram_tensor("sall", [3, 1875], F32, kind="ExternalInput")
    t_sst = nc.dram_tensor("sst", [3, 243], F32, kind="ExternalInput")
    t_s48 = nc.dram_tensor("s48", [3, 768], F32, kind="ExternalInput")
    t_ident = nc.dram_tensor("ident", [100, 100], F32, kind="ExternalInput")
    t_aux = nc.dram_tensor("aux", [1, 79], F32, kind="ExternalInput")
    t_x0w = nc.dram_tensor("x0w", [48, NF], F32, kind="ExternalInput")
    t_out96a = nc.dram_tensor("out96a", [112, CHUNK], BF16, kind="ExternalOutput")
    t_out96b = nc.dram_tensor("out96b", [112, CHUNK], BF16, kind="ExternalOutput")
    t_out2 = nc.dram_tensor("out2", [48, 16], BF16, kind="ExternalOutput")

    with tile.TileContext(nc) as tc:
        with tc.tile_pool(name="sb", bufs=1) as sb, \
             tc.tile_pool(name="sbc", bufs=4) as sbc, \
             tc.tile_pool(name="psA", bufs=1, space="PSUM") as psA, \
             tc.tile_pool(name="psB", bufs=1, space="PSUM") as psB, \
             tc.tile_pool(name="psF", bufs=1, space="PSUM") as psF:

            # ---- loads (all issued up front; none mid-chain)
            xp1_sb = sb.tile([76, 1024], F32)
            xp2_sb = sb.tile([76, 1024], F32)
            w1a_sb = sb.tile([76, 3], F32)
            w1b_sb = sb.tile([76, 3], F32)
            keysA_sb = sb.tile([128, 1200], FP16)
            keysB_sb = sb.tile([128, 1200], FP16)
            vals_sb = sb.tile([100, 225], F32)
            sall_sb = sb.tile([3, 1875], F32)
            sst_sb = sb.tile([3, 243], F32)
            s48_sb = sb.tile([3, 768], F32)
            ident_sb = sb.tile([100, 100], F32)
            aux_sb = sb.tile([1, 79], F32)
            x0w_sb = sb.tile([48, NF], F32)
            for eng, tdst, tsrc in [
                    (nc.sync, xp1_sb, t_xp1),
                    (nc.gpsimd, w1a_sb, t_w1a),
                    (nc.scalar, keysA_sb, t_keysA),
                    (nc.gpsimd, ident_sb, t_ident),
                    (nc.scalar, keysB_sb, t_keysB),
                    (nc.sync, xp2_sb, t_xp2),
                    (nc.gpsimd, vals_sb, t_vals),
                    (nc.scalar, sall_sb, t_sall),
                    (nc.gpsimd, w1b_sb, t_w1b),
                    (nc.scalar, s48_sb, t_s48),
                    (nc.gpsimd, sst_sb, t_sst),
                    (nc.scalar, aux_sb, t_aux),
                    (nc.sync, x0w_sb, t_x0w)]:
                eng.dma_start(tdst[:], tsrc[:])

            ones100 = sb.tile([1, 100], F32)
            nc.gpsimd.memset(ones100[:], 1.0)
            out2_sb = sb.tile([48, 16], BF16)
            nc.gpsimd.memset(out2_sb[:], 0.0)

            # ---- elementwise input prep (2x-1), chunked for pipelining
            xa = sb.tile([76, 1024], F32)
            for xh in range(4):
                nc.vector.tensor_scalar(xa[:, xh * 256:(xh + 1) * 256],
                                        xp1_sb[:, xh * 256:(xh + 1) * 256],
                                        2.0, -1.0,
                                        op0=ALU.mult, op1=ALU.add)
            kvA = keysA_sb.rearrange("p (cc k) -> p cc k", k=100)
            kvB = keysB_sb.rearrange("p (cc k) -> p cc k", k=100)

            # ---------------- key/attention stage (shared emitter)
            # returns normalized kernel row as [3, 75] (rows ic / c,
            # cols (k_t, k_tx, other-channel))
            def key_stage(xaug_sb, w_sb, tag):
                # conv keys: one batched [128, 24] PSUM tile
                pk = psA.tile([128, 24], F32, tag="pk")
                for m in range(8):
                    nc.tensor.matmul(pk[:, m * 3:m * 3 + 3],
                                     xaug_sb[:, m * 128:(m + 1) * 128],
                                     w_sb[:], start=True, stop=True,
                                     skip_group_check=True)
                # sigmoid(x) = 0.5*tanh(x/2) + 0.5  (tanh shares exp's table)
                th = sbc.tile([128, 24], F32, tag="th")
                nc.scalar.activation(th[:], pk[:], AF.Tanh, scale=0.5)
                keyT = sb.tile([128, 24], FP16, tag=f"keyT{tag}")
                nc.vector.tensor_scalar(keyT[:], th[:], 0.5, 0.5,
                                        op0=ALU.mult, op1=ALU.add)
                # logits column via one accumulated contraction pass
                lc0 = psB.tile([100, 1], F32, tag="acc")
                for oc in range(3):
                    for m in range(8):
                        cc = oc * 8 + m
                        kvh = kvA[:, cc, :] if cc < 12 else kvB[:, cc - 12, :]
                        nc.tensor.matmul(
                            lc0[:], kvh,
                            keyT[:, m * 3 + oc:m * 3 + oc + 1],
                            start=(cc == 0), stop=(cc == 23))
                lc0_sb = sb.tile([100, 1], F32, tag=f"lc0{tag}")
                nc.vector.tensor_copy(lc0_sb[:], lc0[:])
                # row view via PE transpose (matmul against identity)
                lrT = psA.tile([1, 100], F32, tag="pk")
                nc.tensor.matmul(lrT[:], lc0_sb[:], ident_sb[:],
                                 start=True, stop=True)
                mx = sb.tile([1, 1], F32, tag=f"mx{tag}")
                nc.vector.reduce_max(mx[:], lrT[:], axis=AX.X)
                negm = sb.tile([1, 1], F32, tag=f"negm{tag}")
                nc.vector.tensor_scalar_mul(negm[:], mx[:], -1.0)
                ex = sb.tile([1, 100], F32, tag=f"ex{tag}")
                Z = sb.tile([1, 1], F32, tag=f"Z{tag}")
                nc.scalar.activation(ex[:], lrT[:], AF.Exp, bias=negm[:],
                                     accum_out=Z[:])
                rz = sb.tile([1, 1], F32, tag=f"rz{tag}")
                nc.vector.reciprocal(rz[:], Z[:])
                # attention column [100,1] via 1-partition PE transpose;
                # rhs = 1/Z folds the softmax normalization into the same op
                exc = psA.tile([100, 1], F32, tag="pk")
                nc.tensor.matmul(exc[:], ex[:], rz[:],
                                 start=True, stop=True)
                exc_sb = sb.tile([100, 1], F32, tag=f"exc{tag}")
                nc.vector.tensor_copy(exc_sb[:], exc[:])
                # kernel row as [75, 3] (column writes are offset-free),
                # then PE-transpose to the consumer layout [3, 75]
                krT = psB.tile([75, 3], F32, tag="acc")
                for ic in range(3):
                    nc.tensor.matmul(krT[:, ic:ic + 1],
                                     vals_sb[:, ic * 75:(ic + 1) * 75],
                                     exc_sb[:], start=True, stop=True,
                                     skip_group_check=True)
                krT_sb = sb.tile([75, 3], F32, tag=f"krT{tag}")
                nc.vector.tensor_copy(krT_sb[:], krT[:])
                krp = psA.tile([3, 75], F32, tag="pk")
                nc.tensor.matmul(krp[:], krT_sb[:], ident_sb[0:75, 0:75],
                                 start=True, stop=True)
                kresh = sb.tile([3, 75], F32, tag=f"kresh{tag}")
                nc.vector.tensor_copy(kresh[:], krp[:])
                return kresh

            # ---------------- stage K1
            k1resh = key_stage(xa, w1a_sb, "1")

            xm2 = sb.tile([76, 1024], F32)
            for xh in range(2):
                nc.vector.tensor_scalar(xm2[:, xh * 512:(xh + 1) * 512],
                                        xp2_sb[:, xh * 512:(xh + 1) * 512],
                                        2.0, -1.0,
                                        op0=ALU.mult, op1=ALU.add)

            # ---------------- T'' via 25 accumulated selection matmuls
            # T''[(ic,u,v),(ky,kx,c)] = k1[ic,c,ky+4-2u,kx+4-2v]
            k1rv = k1resh.rearrange("ic (kt ktx c) -> ic kt ktx c",
                                    kt=5, ktx=5)
            tpps = psB.tile([75, 75], F32, tag="bld")
            tppsv = tpps.rearrange("p (ky kx c) -> p ky kx c", ky=5, kx=5)
            uvs = [(2, 2)] + [(u, v) for u in range(5) for v in range(5)
                              if (u, v) != (2, 2)]
            for i, (u, v) in enumerate(uvs):
                klo, khi = max(0, 2 * u - 4), min(4, 2 * u)
                xlo, xhi = max(0, 2 * v - 4), min(4, 2 * v)
                nc.tensor.matmul(
                    tppsv[:, klo:khi + 1, xlo:xhi + 1, :],
                    sall_sb[:, (u * 5 + v) * 75:(u * 5 + v + 1) * 75],
                    k1rv[:, klo + 4 - 2 * u:khi + 5 - 2 * u,
                         xlo + 4 - 2 * v:xhi + 5 - 2 * v, :],
                    start=(i == 0), stop=(i == len(uvs) - 1),
                    skip_group_check=True)
            tpp_sb = sb.tile([75, 75], F32)
            nc.vector.tensor_copy(tpp_sb[:], tpps[:])

            # ---------------- compose K2 weights: WK2 = [T'' @ w1b75 ; b]
            ptp = psB.tile([75, 75], F32, tag="bld")
            nc.tensor.matmul(ptp[:], tpp_sb[:], ident_sb[0:75, 0:75],
                             start=True, stop=True)
            tppT_sb = sb.tile([75, 75], F32)
            nc.vector.tensor_copy(tppT_sb[:], ptp[:])
            pwk = psB.tile([76, 3], F32, tag="bld")
            # rank-1 bias row first (start zeroes all 76 rows), then the
            # weight part accumulates rows 0..74
            nc.tensor.matmul(pwk[:], aux_sb[:, 3:79], aux_sb[:, 0:3],
                             start=True, stop=False, skip_group_check=True)
            nc.tensor.matmul(pwk[0:75, :], tppT_sb[:], w1b_sb[0:75, :],
                             start=False, stop=True, skip_group_check=True)
            wk2_sb = sb.tile([76, 3], F32)
            nc.vector.tensor_copy(wk2_sb[:], pwk[:])

            # xwm prep (needed only by stage F; DVE is free here)
            xwm = sb.tile([48, NF], BF16)
            for xc in range(4):
                lo = xc * ((NF + 3) // 4)
                hi = min(NF, (xc + 1) * ((NF + 3) // 4))
                nc.vector.tensor_scalar(xwm[:, lo:hi], x0w_sb[:, lo:hi],
                                        2.0, -1.0,
                                        op0=ALU.mult, op1=ALU.add)

            # ---------------- T' variants [48, 27] via selection matmuls
            # (emitted before K2 so the PE fills K2's latency gaps)
            tpv_sb = sb.tile([48, 108], F32)
            for dvi, (er_, ec_) in enumerate([(1, 1), (1, 2), (2, 1),
                                              (2, 2)]):
                def blocks(e):
                    bl = []
                    for u in range(4):
                        lo, hi = max(0, 2 * u - 2 - e), min(2, 2 * u + 2 - e)
                        if lo <= hi:
                            bl.append((u, lo, hi))
                    return bl
                ub, vb = blocks(er_), blocks(ec_)
                ub.sort(key=lambda b: -(b[2] - b[1]))
                vb.sort(key=lambda b: -(b[2] - b[1]))
                tps = psB.tile([48, 27], F32, tag="bld")
                tpsv = tps.rearrange("p (c s t) -> p s t c", s=3, t=3)
                nbl = len(ub) * len(vb)
                j = 0
                for u, slo, shi in ub:
                    ktlo = er_ + 2 + slo - 2 * u
                    for v, tlo, thi in vb:
                        ktxlo = ec_ + 2 + tlo - 2 * v
                        j += 1
                        nc.tensor.matmul(
                            tpsv[:, slo:shi + 1, tlo:thi + 1, :],
                            s48_sb[:, (u * 4 + v) * 48:(u * 4 + v + 1) * 48],
                            k1rv[:, ktlo:ktlo + shi - slo + 1,
                                 ktxlo:ktxlo + thi - tlo + 1, :],
                            start=(j == 1), stop=(j == nbl),
                            skip_group_check=True)
                nc.vector.tensor_copy(
                    tpv_sb[:, dvi * 27:(dvi + 1) * 27], tps[:])
            # transposed variants [27, 48] for the W composition
            tpvT_sb = sb.tile([27, 192], F32)
            for dvi in range(4):
                ptt = psA.tile([27, 48], F32, tag="pwf")
                nc.tensor.matmul(ptt[:],
                                 tpv_sb[:, dvi * 27:(dvi + 1) * 27],
                                 ident_sb[0:48, 0:48], start=True, stop=True)
                nc.vector.tensor_copy(
                    tpvT_sb[:, dvi * 48:(dvi + 1) * 48], ptt[:])

            # ---------------- stage K2
            k2resh = key_stage(xm2, wk2_sb, "2")

            # ---------------- F variants via selection matmuls
            srange = {0: (0, 3, 2), 1: (0, 2, 1), 2: (1, 3, 2)}
            k2rv = k2resh.rearrange("c (ky kx o) -> c ky kx o", ky=5, kx=5)
            fps = psB.tile([27, 27], F32, tag="bld")
            for vr_ in range(3):
                slo, shi, fy = srange[vr_]
                for vc_ in range(3):
                    tlo, thi, fx = srange[vc_]
                    vi = vr_ * 3 + vc_
                    sts = [(s, t) for s in range(slo, shi)
                           for t in range(tlo, thi)]
                    for j, (s, t) in enumerate(sts):
                        nc.tensor.matmul(
                            fps[:, vi * 3:vi * 3 + 3],
                            sst_sb[:, (s * 3 + t) * 27:(s * 3 + t + 1) * 27],
                            k2rv[:, fy + 2 - 2 * s, fx + 2 - 2 * t, :],
                            start=(j == 0), stop=(j == len(sts) - 1),
                            skip_group_check=True)
            f_sb = sb.tile([27, 27], F32)
            nc.vector.tensor_copy(f_sb[:], fps[:])

            # ---------------- W_k = T'varT.T @ F_var  -> [48, 48]
            dd, df = st['dtype_delta'], st['dtype_f']
            used = st['used']
            w_sb = sb.tile([48, 48], BF16)
            pwall = psA.tile([48, 48], F32, tag="pwf")
            for ki, k in enumerate(used):
                ta, tb = k // 5, k % 5
                dvi = dd[ta] * 2 + dd[tb]
                fvi = df[ta] * 3 + df[tb]
                nc.tensor.matmul(pwall[:, ki * 3:ki * 3 + 3],
                                 tpvT_sb[:, dvi * 48:(dvi + 1) * 48],
                                 f_sb[:, fvi * 3:fvi * 3 + 3],
                                 start=True, stop=True, skip_group_check=True)
            nc.vector.tensor_copy(w_sb[:], pwall[:])

            # ---------------- stage F: tmp = W^T @ xwm, chunked fp32r
            # chunk parity stacks at partition offset 64 (PE tile-position
            # rule); rows 48..63 are memset so the full-tile sigmoid stays
            # finite
            bankA = psF.tile([112, CHUNK], F32, tag="fa")
            bankB = psF.tile([112, CHUNK], F32, tag="fb")
            bankC = psF.tile([48, 16], F32, tag="fc")
            nc.vector.memset(bankA[32:64, :], 0.0)
            nc.vector.memset(bankB[32:64, :], 0.0)
            out96 = sb.tile([112, 1024], BF16)
            for ch in range(4):
                bank = bankA if ch < 2 else bankB
                po = 64 * (ch % 2)
                nc.tensor.matmul(bank[po:po + 48, :], w_sb[:],
                                 xwm[:, ch * CHUNK:(ch + 1) * CHUNK],
                                 start=True, stop=True,
                                 skip_group_check=True)
            nrem = NF - 4 * CHUNK
            nc.tensor.matmul(bankC[:, 0:nrem], w_sb[:],
                             xwm[:, 4 * CHUNK:NF],
                             start=True, stop=True, skip_group_check=True)
            nc.scalar.activation(out96[:, 0:CHUNK], bankA[:], AF.Sigmoid)
            nc.sync.dma_start(t_out96a[:], out96[:, 0:CHUNK])
            nc.scalar.activation(out96[:, CHUNK:1024], bankB[:], AF.Sigmoid)
            nc.scalar.activation(out2_sb[:, 0:nrem], bankC[:, 0:nrem],
                                 AF.Sigmoid)
            nc.gpsimd.dma_start(t_out2[:], out2_sb[:])
            nc.scalar.dma_start(t_out96b[:], out96[:, CHUNK:1024])
    nc.compile()
    return nc


# ---------------------------------------------------------------------------
# entry point
# ---------------------------------------------------------------------------

def _run(ins, trace=False):
    from concourse.bass_utils import run_bass_kernel_spmd
    if 'nc' not in _nc_cache:
        _nc_cache['nc'] = _build_nc()
    nc = _nc_cache['nc']
    d, x0w = _prep(ins, _ST)
    in_maps = [{**d, "x0w": x0w[c]} for c in range(NCORES)]
    res = run_bass_kernel_spmd(nc, in_maps, core_ids=list(range(NCORES)),
                               trace=trace)
    return res


def _assemble(results):
    st = _ST
    kidx = st['kidx']
    s = np.arange(NF)
    chunk = s // CHUNK
    final = np.zeros((3, 128, 128), np.float32)
    for c in range(NCORES):
        r = results[c]
        o96 = np.concatenate([np.asarray(r["out96a"], np.float32),
                              np.asarray(r["out96b"], np.float32)], axis=1)
        o2 = np.asarray(r["out2"], np.float32)
        vals = np.empty((3, NF), np.float32)
        for ch_ in range(3):
            row = 3 * kidx + ch_
            in96 = chunk < 4
            vals[ch_, in96] = o96[64 * (chunk[in96] % 2) + row[in96],
                                  CHUNK * (chunk[in96] // 2) + s[in96] % CHUNK]
            vals[ch_, ~in96] = o2[row[~in96], s[~in96] - 4 * CHUNK]
        pix = st['pix_of_slot'][c]
        valid = pix >= 0
        final[:, pix[valid] // 128, pix[valid] % 128] = vals[:, valid]
    return final[None]


def kernel(**inputs) -> np.ndarray:
    res = _run(inputs)
    return _assemble(res.results)


# revision 24
# speedup vs baseline: 1.0978x; 1.0022x over previous
"""Trainium2 Bass kernel for nn_Net_52218212384916.

Math identical to the previous revision (sample-point collapse of the two
conv_transpose stages; see git history / kernel_baseline.py).  This revision
restructures the device program for latency:

  - no mid-program DMAs: kernel-row reshapes ([1,225] -> [3,75]) are done on
    the PE via an ex-column transpose + 3 value matmuls at partition offsets
  - softmax uses the row form only, normalizes ex by 1/Z up front, so the
    downstream kernel rows need no deferred scaling
  - the 8 per-block key sigmoids are batched into one [128,24] PSUM tile and
    one tanh activation (sigmoid(x) = 0.5 tanh(x/2) + 0.5; tanh shares the
    ACT table with exp, so the only table switch is the final sigmoid, which
    lands in an idle ACT window)
  - stage F is 5 chunked fp32r matmuls (1 cycle/row at >=256 cols) into two
    stacked [96,512] PSUM banks + one small bank, 3 sigmoid passes, 2 output
    DMAs
"""
import numpy as np

H0 = 1024
S1 = (H0 - 5) // 2 + 1          # 510   conv1 output size
O1 = 2 * H0 - 1                 # 2047  out1 size
S2 = (O1 - 5) // 2 + 1          # 1022  conv2 output size
O2 = 2 * O1 - 1                 # 4093  out2 size
NCORES = 8
CHUNK = 512

_nc_cache = {}


# ---------------------------------------------------------------------------
# static structure (shapes only)
# ---------------------------------------------------------------------------

def _static():
    st = {}
    r1 = np.arange(32) * S1 // 32
    r2 = np.arange(32) * S2 // 32
    rf = np.arange(128) * O2 // 128
    a = -(-(rf - 2) // 2)            # first contributing out1 row
    gy = -(-(a - 2) // 2)            # first contributing x0 row
    e = a - 2 * gy                   # phase in {1,2}
    delta = (e == 2).astype(int)
    f = np.where(rf % 2 == 0, 2, 1)
    dim_type = np.empty(128, int)
    tmap = {(0, 2): 0, (1, 1): 1, (1, 2): 2, (0, 1): 3}
    for i in range(128):
        dim_type[i] = 4 if i == 0 else tmap[(delta[i], f[i])]
    st.update(r1=r1, r2=r2, rf=rf, a=a, gy=gy, dim_type=dim_type)
    st['dtype_delta'] = {0: 0, 1: 1, 2: 1, 3: 0, 4: 0}
    st['dtype_f'] = {0: 0, 1: 1, 2: 0, 3: 1, 4: 2}

    cls = dim_type[:, None] * 5 + dim_type[None, :]
    order = np.argsort(cls.ravel(), kind='stable')
    counts = np.bincount(cls.ravel(), minlength=25)
    Q = -(-counts // NCORES)
    offs = np.concatenate([[0], np.cumsum(Q)]).astype(int)
    NF = int(offs[-1])
    used = [k for k in range(25) if counts[k] > 0]
    pix_of_slot = -np.ones((NCORES, NF), np.int64)
    cstart = np.concatenate([[0], np.cumsum(counts)])
    for k in used:
        plist = order[cstart[k]:cstart[k + 1]]
        for c in range(NCORES):
            seg = plist[c * Q[k]:(c + 1) * Q[k]]
            pix_of_slot[c, offs[k]:offs[k] + len(seg)] = seg
    st.update(counts=counts, Q=Q, offs=offs, NF=NF, used=used,
              pix_of_slot=pix_of_slot)
    # slot -> used-class index (for output row addressing)
    kidx = np.zeros(NF, int)
    for ki, k in enumerate(used):
        kidx[offs[k]:offs[k] + Q[k]] = ki
    st['kidx'] = kidx
    return st


_ST = _static()
NF = _ST['NF']
NCH = -(-NF // CHUNK)            # 5 column chunks in stage F


# ---------------------------------------------------------------------------
# host-side gathers (raw values only; OOB -> 0.5)
# ---------------------------------------------------------------------------

def _gather_patches(img, row0s, col0s, n):
    C, H, W = img.shape
    R = row0s[:, None] + np.arange(n)[None, :]
    Cc = col0s[:, None] + np.arange(n)[None, :]
    vr, vc = (R >= 0) & (R < H), (Cc >= 0) & (Cc < W)
    Rc, Ccc = np.clip(R, 0, H - 1), np.clip(Cc, 0, W - 1)
    out = img[:, Rc[:, None, :, None], Ccc[None, :, None, :]]
    mask = vr[:, None, :, None] & vc[None, :, None, :]
    out = np.where(mask[None], out, np.float32(0.5))
    C_, NI, NJ, n_, _ = out.shape
    return np.ascontiguousarray(
        out.transpose(0, 3, 4, 1, 2).reshape(C_ * n_ * n_, NI * NJ), np.float32)


def _prep(ins, st):
    img = np.asarray(ins['input'], np.float32)[0]
    r1, r2, gy = st['r1'], st['r2'], st['gy']
    d = {}
    xp1 = _gather_patches(img, 2 * r1, 2 * r1, 5)              # [75,1024]
    d['xp1aug'] = np.concatenate(
        [xp1, np.full((1, 1024), 1.0, np.float32)], 0)         # [76,1024]
    x0p2 = _gather_patches(img, r2 - 1, r2 - 1, 5)             # [75,1024]
    d['x0p2'] = np.concatenate(
        [x0p2, np.full((1, 1024), 1.0, np.float32)], 0)        # [76,1024]
    w1 = np.asarray(ins['lk1_conv_w'], np.float32)             # [oc,ic,5,5]
    b1 = np.asarray(ins['lk1_conv_b'], np.float32)
    # K1 weights: rows (ic,ky,kx)+bias ; K2 weights: rows (ky,kx,c)+bias
    wa = w1.transpose(1, 2, 3, 0).reshape(75, 3)
    wb = w1.transpose(2, 3, 1, 0).reshape(75, 3)
    d['w1a'] = np.concatenate([wa, b1[None]], 0).astype(np.float32)
    d['w1b'] = np.concatenate([wb, b1[None]], 0).astype(np.float32)
    keys = np.asarray(ins['lk1_keys'], np.float32)             # [100,3072]
    keysR = np.ascontiguousarray(
        keys.T.reshape(24, 128, 100).transpose(1, 0, 2), np.float32
    ).reshape(128, 2400)
    d['keysA'] = np.ascontiguousarray(keysR[:, :1200]).astype(np.float16)
    d['keysB'] = np.ascontiguousarray(keysR[:, 1200:]).astype(np.float16)
    # values with columns permuted (in,out,ky,kx) -> (in,ky,kx,out)
    vals = np.asarray(ins['lk1_values'], np.float32)
    d['valsP'] = np.ascontiguousarray(
        vals.reshape(100, 3, 3, 5, 5).transpose(0, 1, 3, 4, 2)
    ).reshape(100, 225)

    # T'' selection [3, 25*75]: sall[ic, uv*75 + ic*25 + uv] = 1
    sall = np.zeros((3, 1875), np.float32)
    for ic in range(3):
        for uv in range(25):
            sall[ic, uv * 75 + ic * 25 + uv] = 1.0
    d['sall'] = sall
    # T' selection [3, 16*48]: s48[ic, uv*48 + uv*3 + ic] = 1  (rows (u,v,ic))
    s48 = np.zeros((3, 768), np.float32)
    for ic in range(3):
        for uv in range(16):
            s48[ic, uv * 48 + uv * 3 + ic] = 1.0
    d['s48'] = s48
    # F (s,t) selection [3, 9*27]: sst[c, st*27 + c*9 + st] = 1
    sst = np.zeros((3, 243), np.float32)
    for c in range(3):
        for stx in range(9):
            sst[c, stx * 27 + c * 9 + stx] = 1.0
    d['sst'] = sst
    d['ident'] = np.eye(100, dtype=np.float32)
    # aux row: [bias(3) | e75 one-hot(76)] for the wk2 bias outer-product
    aux = np.zeros((1, 79), np.float32)
    aux[0, 0:3] = b1
    aux[0, 3 + 75] = 1.0
    d['aux'] = aux

    # stage F windows, per core
    pix = st['pix_of_slot']
    uu = np.arange(4)
    x0w = []
    for c in range(NCORES):
        p = pix[c]
        ii, jj = p // 128, p % 128
        R = gy[np.clip(ii, 0, 127)][:, None] + uu[None, :]
        Cc = gy[np.clip(jj, 0, 127)][:, None] + uu[None, :]
        ok = (p >= 0)[:, None]
        vr = (R >= 0) & (R < H0) & ok
        vc = (Cc >= 0) & (Cc < H0) & ok
        Rc, Ccc = np.clip(R, 0, H0 - 1), np.clip(Cc, 0, H0 - 1)
        g = img[:, Rc[:, :, None], Ccc[:, None, :]]            # [3,NF,4,4]
        m = vr[:, :, None] & vc[:, None, :]
        g = np.where(m[None], g, np.float32(0.5))
        # row order (u, v, ic) to match M4T/W layout
        x0w.append(np.ascontiguousarray(
            g.transpose(2, 3, 0, 1).reshape(48, NF), np.float32))
    return d, x0w


# ---------------------------------------------------------------------------
# device program
# ---------------------------------------------------------------------------

def _build_nc():
    import concourse.bacc as bacc
    import concourse.tile as tile
    from concourse import mybir

    F32 = mybir.dt.float32
    F32R = mybir.dt.float32r
    BF16 = mybir.dt.bfloat16
    FP16 = mybir.dt.float16
    AF = mybir.ActivationFunctionType
    ALU = mybir.AluOpType
    AX = mybir.AxisListType
    st = _ST

    nc = bacc.Bacc("TRN2", target_bir_lowering=False, debug=False)
    t_xp1 = nc.dram_tensor("xp1aug", [76, 1024], F32, kind="ExternalInput")
    t_xp2 = nc.dram_tensor("x0p2", [76, 1024], F32, kind="ExternalInput")
    t_w1a = nc.dram_tensor("w1a", [76, 3], F32, kind="ExternalInput")
    t_w1b = nc.dram_tensor("w1b", [76, 3], F32, kind="ExternalInput")
    t_keysA = nc.dram_tensor("keysA", [128, 1200], FP16, kind="ExternalInput")
    t_keysB = nc.dram_tensor("keysB", [128, 1200], FP16, kind="ExternalInput")
    t_vals = nc.dram_tensor("valsP", [100, 225], F32, kind="ExternalInput")
    t_sall = nc.dram_tensor("sall", [3, 1875], F32, kind="ExternalInput")
    t_sst = nc.dram_tensor("sst", [3, 243], F32, kind="ExternalInput")
    t_s48 = nc.dram_tensor("s48", [3, 768], F32, kind="ExternalInput")
    t_ident = nc.dram_tensor("ident", [100, 100], F32, kind="ExternalInput")
    t_aux = nc.dram_tensor("aux", [1, 79], F32, kind="ExternalInput")
    t_x0w = nc.dram_tensor("x0w", [48, NF], F32, kind="ExternalInput")
    t_out96a = nc.dram_tensor("out96a", [112, CHUNK], BF16, kind="ExternalOutput")
    t_out96b = nc.dram_tensor("out96b", [112, 1040 - CHUNK], BF16,
                              kind="ExternalOutput")

    with tile.TileContext(nc) as tc:
        with tc.tile_pool(name="sb", bufs=1) as sb, \
             tc.tile_pool(name="sbc", bufs=4) as sbc, \
             tc.tile_pool(name="psA", bufs=1, space="PSUM") as psA, \
             tc.tile_pool(name="psB", bufs=1, space="PSUM") as psB, \
             tc.tile_pool(name="psF", bufs=1, space="PSUM") as psF:

            # ---- loads (all issued up front; none mid-chain)
            xp1_sb = sb.tile([76, 1024], F32)
            xp2_sb = sb.tile([76, 1024], F32)
            w1a_sb = sb.tile([76, 3], F32)
            w1b_sb = sb.tile([76, 3], F32)
            keysA_sb = sb.tile([128, 1200], FP16)
            keysB_sb = sb.tile([128, 1200], FP16)
            vals_sb = sb.tile([100, 225], F32)
            sall_sb = sb.tile([3, 1875], F32)
            sst_sb = sb.tile([3, 243], F32)
            s48_sb = sb.tile([3, 768], F32)
            ident_sb = sb.tile([100, 100], F32)
            aux_sb = sb.tile([1, 79], F32)
            x0w_sb = sb.tile([48, NF], F32)
            for eng, tdst, tsrc in [
                    (nc.sync, xp1_sb, t_xp1),
                    (nc.gpsimd, w1a_sb, t_w1a),
                    (nc.scalar, keysA_sb, t_keysA),
                    (nc.gpsimd, ident_sb, t_ident),
                    (nc.scalar, keysB_sb, t_keysB),
                    (nc.sync, xp2_sb, t_xp2),
                    (nc.gpsimd, vals_sb, t_vals),
                    (nc.scalar, sall_sb, t_sall),
                    (nc.gpsimd, w1b_sb, t_w1b),
                    (nc.scalar, s48_sb, t_s48),
                    (nc.gpsimd, sst_sb, t_sst),
                    (nc.scalar, aux_sb, t_aux),
                    (nc.sync, x0w_sb, t_x0w)]:
                eng.dma_start(tdst[:], tsrc[:])

            ones100 = sb.tile([1, 100], F32)
            nc.gpsimd.memset(ones100[:], 1.0)
            out96 = sb.tile([112, 1040], BF16)
            nc.gpsimd.memset(out96[:, 1024:1040], 0.0)

            # ---- elementwise input prep (2x-1), chunked for pipelining
            xa = sb.tile([76, 1024], F32)
            for xh in range(4):
                nc.vector.tensor_scalar(xa[:, xh * 256:(xh + 1) * 256],
                                        xp1_sb[:, xh * 256:(xh + 1) * 256],
                                        2.0, -1.0,
                                        op0=ALU.mult, op1=ALU.add)
            kvA = keysA_sb.rearrange("p (cc k) -> p cc k", k=100)
            kvB = keysB_sb.rearrange("p (cc k) -> p cc k", k=100)

            # ---------------- key/attention stage (shared emitter)
            # returns normalized kernel row as [3, 75] (rows ic / c,
            # cols (k_t, k_tx, other-channel))
            def key_stage(xaug_sb, w_sb, tag):
                # conv keys: one batched [128, 24] PSUM tile
                pk = psA.tile([128, 24], F32, tag="pk")
                for m in range(8):
                    nc.tensor.matmul(pk[:, m * 3:m * 3 + 3],
                                     xaug_sb[:, m * 128:(m + 1) * 128],
                                     w_sb[:], start=True, stop=True,
                                     skip_group_check=True)
                # sigmoid(x) = 0.5*tanh(x/2) + 0.5  (tanh shares exp's table)
                th = sbc.tile([128, 24], F32, tag="th")
                nc.scalar.activation(th[:], pk[:], AF.Tanh, scale=0.5)
                keyT = sb.tile([128, 24], FP16, tag=f"keyT{tag}")
                nc.vector.tensor_scalar(keyT[:], th[:], 0.5, 0.5,
                                        op0=ALU.mult, op1=ALU.add)
                # logits column via one accumulated contraction pass
                lc0 = psB.tile([100, 1], F32, tag="acc")
                for oc in range(3):
                    for m in range(8):
                        cc = oc * 8 + m
                        kvh = kvA[:, cc, :] if cc < 12 else kvB[:, cc - 12, :]
                        nc.tensor.matmul(
                            lc0[:], kvh,
                            keyT[:, m * 3 + oc:m * 3 + oc + 1],
                            start=(cc == 0), stop=(cc == 23))
                lc0_sb = sb.tile([100, 1], F32, tag=f"lc0{tag}")
                nc.vector.tensor_copy(lc0_sb[:], lc0[:])
                # row view via PE transpose (matmul against identity)
                lrT = psA.tile([1, 100], F32, tag="pk")
                nc.tensor.matmul(lrT[:], lc0_sb[:], ident_sb[:],
                                 start=True, stop=True)
                mx = sb.tile([1, 1], F32, tag=f"mx{tag}")
                nc.vector.reduce_max(mx[:], lrT[:], axis=AX.X)
                negm = sb.tile([1, 1], F32, tag=f"negm{tag}")
                nc.vector.tensor_scalar_mul(negm[:], mx[:], -1.0)
                ex = sb.tile([1, 100], F32, tag=f"ex{tag}")
                Z = sb.tile([1, 1], F32, tag=f"Z{tag}")
                nc.scalar.activation(ex[:], lrT[:], AF.Exp, bias=negm[:],
                                     accum_out=Z[:])
                rz = sb.tile([1, 1], F32, tag=f"rz{tag}")
                nc.vector.reciprocal(rz[:], Z[:])
                # attention column [100,1] via 1-partition PE transpose;
                # rhs = 1/Z folds the softmax normalization into the same op
                exc = psA.tile([100, 1], F32, tag="pk")
                nc.tensor.matmul(exc[:], ex[:], rz[:],
                                 start=True, stop=True)
                exc_sb = sb.tile([100, 1], F32, tag=f"exc{tag}")
                nc.vector.tensor_copy(exc_sb[:], exc[:])
                # kernel row as [75, 3] (column writes are offset-free),
                # then PE-transpose to the consumer layout [3, 75]
                krT = psB.tile([75, 3], F32, tag="acc")
                for ic in range(3):
                    nc.tensor.matmul(krT[:, ic:ic + 1],
                                     vals_sb[:, ic * 75:(ic + 1) * 75],
                                     exc_sb[:], start=True, stop=True,
                                     skip_group_check=True)
                krT_sb = sb.tile([75, 3], F32, tag=f"krT{tag}")
                nc.vector.tensor_copy(krT_sb[:], krT[:])
                krp = psA.tile([3, 75], F32, tag="pk")
                nc.tensor.matmul(krp[:], krT_sb[:], ident_sb[0:75, 0:75],
                                 start=True, stop=True)
                kresh = sb.tile([3, 75], F32, tag=f"kresh{tag}")
                nc.vector.tensor_copy(kresh[:], krp[:])
                return kresh

            # ---------------- stage K1
            k1resh = key_stage(xa, w1a_sb, "1")

            xm2 = sb.tile([76, 1024], F32)
            for xh in range(2):
                nc.vector.tensor_scalar(xm2[:, xh * 512:(xh + 1) * 512],
                                        xp2_sb[:, xh * 512:(xh + 1) * 512],
                                        2.0, -1.0,
                                        op0=ALU.mult, op1=ALU.add)

            # ---------------- T'' via 25 accumulated selection matmuls
            # T''[(ic,u,v),(ky,kx,c)] = k1[ic,c,ky+4-2u,kx+4-2v]
            k1rv = k1resh.rearrange("ic (kt ktx c) -> ic kt ktx c",
                                    kt=5, ktx=5)
            tpps = psB.tile([75, 75], F32, tag="bld")
            tppsv = tpps.rearrange("p (ky kx c) -> p ky kx c", ky=5, kx=5)
            uvs = [(2, 2)] + [(u, v) for u in range(5) for v in range(5)
                              if (u, v) != (2, 2)]
            for i, (u, v) in enumerate(uvs):
                klo, khi = max(0, 2 * u - 4), min(4, 2 * u)
                xlo, xhi = max(0, 2 * v - 4), min(4, 2 * v)
                nc.tensor.matmul(
                    tppsv[:, klo:khi + 1, xlo:xhi + 1, :],
                    sall_sb[:, (u * 5 + v) * 75:(u * 5 + v + 1) * 75],
                    k1rv[:, klo + 4 - 2 * u:khi + 5 - 2 * u,
                         xlo + 4 - 2 * v:xhi + 5 - 2 * v, :],
                    start=(i == 0), stop=(i == len(uvs) - 1),
                    skip_group_check=True)
            tpp_sb = sb.tile([75, 75], F32)
            nc.vector.tensor_copy(tpp_sb[:], tpps[:])

            # ---------------- compose K2 weights: WK2 = [T'' @ w1b75 ; b]
            ptp = psB.tile([75, 75], F32, tag="bld")
            nc.tensor.matmul(ptp[:], tpp_sb[:], ident_sb[0:75, 0:75],
                             start=True, stop=True)
            tppT_sb = sb.tile([75, 75], F32)
            nc.vector.tensor_copy(tppT_sb[:], ptp[:])
            pwk = psB.tile([76, 3], F32, tag="bld")
            # rank-1 bias row first (start zeroes all 76 rows), then the
            # weight part accumulates rows 0..74
            nc.tensor.matmul(pwk[:], aux_sb[:, 3:79], aux_sb[:, 0:3],
                             start=True, stop=False, skip_group_check=True)
            nc.tensor.matmul(pwk[0:75, :], tppT_sb[:], w1b_sb[0:75, :],
                             start=False, stop=True, skip_group_check=True)
            wk2_sb = sb.tile([76, 3], F32)
            nc.vector.tensor_copy(wk2_sb[:], pwk[:])

            # xwm prep (needed only by stage F; DVE is free here)
            xwm = sb.tile([48, NF], BF16)
            for xc in range(4):
                lo = xc * ((NF + 3) // 4)
                hi = min(NF, (xc + 1) * ((NF + 3) // 4))
                nc.vector.tensor_scalar(xwm[:, lo:hi], x0w_sb[:, lo:hi],
                                        2.0, -1.0,
                                        op0=ALU.mult, op1=ALU.add)

            # ---------------- T' variants [48, 27] via selection matmuls
            # (emitted before K2 so the PE fills K2's latency gaps)
            tpv_sb = sb.tile([48, 108], F32)
            for dvi, (er_, ec_) in enumerate([(1, 1), (1, 2), (2, 1),
                                              (2, 2)]):
                def blocks(e):
                    bl = []
                    for u in range(4):
                        lo, hi = max(0, 2 * u - 2 - e), min(2, 2 * u + 2 - e)
                        if lo <= hi:
                            bl.append((u, lo, hi))
                    return bl
                ub, vb = blocks(er_), blocks(ec_)
                ub.sort(key=lambda b: -(b[2] - b[1]))
                vb.sort(key=lambda b: -(b[2] - b[1]))
                tps = psB.tile([48, 27], F32, tag="bld")
                tpsv = tps.rearrange("p (c s t) -> p s t c", s=3, t=3)
                nbl = len(ub) * len(vb)
                j = 0
                for u, slo, shi in ub:
                    ktlo = er_ + 2 + slo - 2 * u
                    for v, tlo, thi in vb:
                        ktxlo = ec_ + 2 + tlo - 2 * v
                        j += 1
                        nc.tensor.matmul(
                            tpsv[:, slo:shi + 1, tlo:thi + 1, :],
                            s48_sb[:, (u * 4 + v) * 48:(u * 4 + v + 1) * 48],
                            k1rv[:, ktlo:ktlo + shi - slo + 1,
                                 ktxlo:ktxlo + thi - tlo + 1, :],
                            start=(j == 1), stop=(j == nbl),
                            skip_group_check=True)
                nc.vector.tensor_copy(
                    tpv_sb[:, dvi * 27:(dvi + 1) * 27], tps[:])
            # transposed variants [27, 48] for the W composition
            tpvT_sb = sb.tile([27, 192], F32)
            for dvi in range(4):
                ptt = psA.tile([27, 48], F32, tag="pwf")
                nc.tensor.matmul(ptt[:],
                                 tpv_sb[:, dvi * 27:(dvi + 1) * 27],
                                 ident_sb[0:48, 0:48], start=True, stop=True)
                nc.vector.tensor_copy(
                    tpvT_sb[:, dvi * 48:(dvi + 1) * 48], ptt[:])

            # ---------------- stage K2
            k2resh = key_stage(xm2, wk2_sb, "2")

            # ---------------- F variants via selection matmuls
            srange = {0: (0, 3, 2), 1: (0, 2, 1), 2: (1, 3, 2)}
            k2rv = k2resh.rearrange("c (ky kx o) -> c ky kx o", ky=5, kx=5)
            fps = psB.tile([27, 27], F32, tag="bld")
            for vr_ in range(3):
                slo, shi, fy = srange[vr_]
                for vc_ in range(3):
                    tlo, thi, fx = srange[vc_]
                    vi = vr_ * 3 + vc_
                    sts = [(s, t) for s in range(slo, shi)
                           for t in range(tlo, thi)]
                    for j, (s, t) in enumerate(sts):
                        nc.tensor.matmul(
                            fps[:, vi * 3:vi * 3 + 3],
                            sst_sb[:, (s * 3 + t) * 27:(s * 3 + t + 1) * 27],
                            k2rv[:, fy + 2 - 2 * s, fx + 2 - 2 * t, :],
                            start=(j == 0), stop=(j == len(sts) - 1),
                            skip_group_check=True)
            f_sb = sb.tile([27, 27], F32)
            nc.vector.tensor_copy(f_sb[:], fps[:])

            # ---------------- W_k = T'varT.T @ F_var  -> [48, 48]
            dd, df = st['dtype_delta'], st['dtype_f']
            used = st['used']
            w_sb = sb.tile([48, 48], BF16)
            pwall = psA.tile([48, 48], F32, tag="pwf")
            for ki, k in enumerate(used):
                ta, tb = k // 5, k % 5
                dvi = dd[ta] * 2 + dd[tb]
                fvi = df[ta] * 3 + df[tb]
                nc.tensor.matmul(pwall[:, ki * 3:ki * 3 + 3],
                                 tpvT_sb[:, dvi * 48:(dvi + 1) * 48],
                                 f_sb[:, fvi * 3:fvi * 3 + 3],
                                 start=True, stop=True, skip_group_check=True)
            nc.vector.tensor_copy(w_sb[:], pwall[:])

            # ---------------- stage F: tmp = W^T @ xwm, chunked fp32r
            # chunk parity stacks at partition offset 64 (PE tile-position
            # rule); rows 48..63 are memset so the full-tile sigmoid stays
            # finite
            bankA = psF.tile([112, CHUNK], F32, tag="fa")
            bankB = psF.tile([112, CHUNK], F32, tag="fb")
            bankC = psF.tile([48, 16], F32, tag="fc")
            nc.vector.memset(bankA[32:64, :], 0.0)
            nc.vector.memset(bankB[32:64, :], 0.0)
            for ch in range(4):
                bank = bankA if ch < 2 else bankB
                po = 64 * (ch % 2)
                nc.tensor.matmul(bank[po:po + 48, :], w_sb[:],
                                 xwm[:, ch * CHUNK:(ch + 1) * CHUNK],
                                 start=True, stop=True,
                                 skip_group_check=True)
            nrem = NF - 4 * CHUNK
            nc.tensor.matmul(bankC[:, 0:nrem], w_sb[:],
                             xwm[:, 4 * CHUNK:NF],
                             start=True, stop=True, skip_group_check=True)
            nc.scalar.activation(out96[0:48, 1024:1024 + nrem],
                                 bankC[:, 0:nrem], AF.Sigmoid)
            nc.scalar.activation(out96[:, 0:CHUNK], bankA[:], AF.Sigmoid)
            nc.sync.dma_start(t_out96a[:], out96[:, 0:CHUNK])
            nc.scalar.activation(out96[:, CHUNK:1024], bankB[:], AF.Sigmoid)
            nc.scalar.dma_start(t_out96b[:], out96[:, CHUNK:1040])
    nc.compile()
    return nc


# ---------------------------------------------------------------------------
# entry point
# ---------------------------------------------------------------------------

def _run(ins, trace=False):
    from concourse.bass_utils import run_bass_kernel_spmd
    if 'nc' not in _nc_cache:
        _nc_cache['nc'] = _build_nc()
    nc = _nc_cache['nc']
    d, x0w = _prep(ins, _ST)
    in_maps = [{**d, "x0w": x0w[c]} for c in range(NCORES)]
    res = run_bass_kernel_spmd(nc, in_maps, core_ids=list(range(NCORES)),
                               trace=trace)
    return res


def _assemble(results):
    st = _ST
    kidx = st['kidx']
    s = np.arange(NF)
    chunk = s // CHUNK
    final = np.zeros((3, 128, 128), np.float32)
    for c in range(NCORES):
        r = results[c]
        o96 = np.concatenate([np.asarray(r["out96a"], np.float32),
                              np.asarray(r["out96b"], np.float32)], axis=1)
        o2 = o96[0:48, 1024:1040]
        vals = np.empty((3, NF), np.float32)
        for ch_ in range(3):
            row = 3 * kidx + ch_
            in96 = chunk < 4
            vals[ch_, in96] = o96[64 * (chunk[in96] % 2) + row[in96],
                                  CHUNK * (chunk[in96] // 2) + s[in96] % CHUNK]
            vals[ch_, ~in96] = o2[row[~in96], s[~in96] - 4 * CHUNK]
        pix = st['pix_of_slot'][c]
        valid = pix >= 0
        final[:, pix[valid] // 128, pix[valid] % 128] = vals[:, valid]
    return final[None]


def kernel(**inputs) -> np.ndarray:
    res = _run(inputs)
    return _assemble(res.results)
